# revision 16
# baseline (speedup 1.0000x reference)
"""Trainium2 Bass kernel for nn_DisentangledHierarchicalEncoder.

Strategy (8 NeuronCores, SPMD, zero collectives):
  Every token's output depends only on its item id (the 4-modal attention is
  within-token), so the host dedups seq_modify to unique items (~32k of 50k),
  shards those across the 8 cores, pre-gathers + L2-normalizes the raw
  features on the host (indices are host-known), and scatters the per-item
  outputs back to tokens afterwards.  Each core runs a dense pipeline over
  its ~4096 item slots in chunks of 512:
      content MLP (1024->1024->256->64), text MLP (768->768->256->64),
      cf linear (64->64), id passthrough,
      per-(item, modality) l2norm + LayerNorm folded into one affine,
      4x4 self-attention (scores via G = 0.125 * wq.T @ wk), mean-pool.
  L1/L2 matmuls run in bf16, the rest in float32r.  Per-item scalars are
  broadcast across partitions with PE outer-products against host-supplied
  pick matrices; partition-stacked pairs (content|text and cf|id) keep every
  engine op on 128 partitions.
"""

import numpy as np

NUM_ITEM = 50000
B, S, D = 1024, 50, 64
DC, DT = 1024, 768
N_CORES = 8
C = 512                           # chunk width (item slots per chunk)
NCH0 = 8                          # default chunks per core (covers ~32.7k uniq)
KC, KT_ = DC // 128, DT // 128    # k-tiles: 8 content, 6 text
LN_EPS = 1e-5
MLP_BF16 = True                   # L1/L2 matmuls in bf16, rest float32r

_CACHE = {}


def _bf():
    if not MLP_BF16:
        return np.float32
    import ml_dtypes
    return ml_dtypes.bfloat16


def _build_consts():
    """Packed attention lhsT constants (modal order 0=c 1=t 2=cf 3=id;
    score row r = 4*m + n)."""
    f32 = np.float32
    # stats: st36 rows 0:4 sums, 32:36 sum-of-squares (32-aligned reads).
    # 4 slices of 36 cols for emb_ct / emb_cfid / sq_ct / sq_cfid.
    hotabw = np.zeros((128, 144), f32)
    for si, (ct, cb) in enumerate([(0, 1), (2, 3), (32, 33), (34, 35)]):
        hotabw[0:64, 36 * si + ct] = 1.0
        hotabw[64:128, 36 * si + cb] = 1.0
    # AB36 [A4 @ 0:4; B4 @ 32:36] -> AB4 rows {0:4 A, 32:36 B, 64:68 A, 96:100 B}
    abrepl = np.zeros((36, 128), f32)
    for j in range(4):
        abrepl[j, j] = 1.0
        abrepl[32 + j, 32 + j] = 1.0
        abrepl[j, 64 + j] = 1.0
        abrepl[32 + j, 96 + j] = 1.0
    # pick: [4, 256]; cols 0:128 broadcast rows 0/1 to halves, cols 128:256 rows 2/3
    pickp = np.zeros((4, 256), f32)
    pickp[0, 0:64] = 1.0
    pickp[1, 64:128] = 1.0
    pickp[2, 128 + 0:128 + 64] = 1.0
    pickp[3, 128 + 64:128 + 128] = 1.0
    # xn broadcasts, row-tiled at 0/32/64/96: (A,ct) (B,ct) (A,cfid) (B,cfid)
    pickq = np.zeros((128, 256), f32)
    pickq[0:4, 0:128] = pickp[:, 0:128]
    pickq[32:36, 0:128] = pickp[:, 0:128]
    pickq[64:68, 128:256] = pickp[:, 128:256]
    pickq[96:100, 128:256] = pickp[:, 128:256]
    # score dots: block b = 2*mu + q (q: 0=ct-pack 1=cfid-pack); top half ->
    # row 4*mu + nu_top(q), bottom -> 4*mu + nu_bot(q).
    nu = {0: (0, 1), 1: (2, 3)}
    hotq = np.zeros((128, 128), f32)
    for mu_i in range(4):
        for q in range(2):
            b = 2 * mu_i + q
            hotq[0:64, 16 * b + 4 * mu_i + nu[q][0]] = 1.0
            hotq[64:128, 16 * b + 4 * mu_i + nu[q][1]] = 1.0
    # softmax sums over n: se row m = sum_n e16 row 4m+n
    sq16 = np.zeros((16, 4), f32)
    for m in range(4):
        for n in range(4):
            sq16[4 * m + n, m] = 1.0
    # replicate rr rows m -> rows 4m+n
    rep16 = np.zeros((4, 16), f32)
    for m in range(4):
        for n in range(4):
            rep16[m, 4 * m + n] = 1.0
    # a4[n] = sum_m w16 row 4m+n, written at rows {n, 32+n}
    nsq = np.zeros((16, 40), f32)
    for m in range(4):
        for n in range(4):
            nsq[4 * m + n, n] = 0.25
            nsq[4 * m + n, 32 + n] = 0.25
    # arp broadcasts, row-tiled at 0/32: ct-pack from a4 rows 0:4,
    # cfid-pack from a4 rows 32:36
    pickr = np.zeros((128, 256), f32)
    pickr[0:4, 0:128] = pickp[:, 0:128]
    pickr[32:36, 128:256] = pickp[:, 128:256]
    return dict(hotabw=hotabw, abrepl=abrepl, pickq=pickq, hotq=hotq,
                sq16=sq16, rep16=rep16, nsq=nsq, pickr=pickr)


def _build_nc(nch=NCH0, n_cores=N_CORES, repeat=1):
    import concourse.bacc as bacc
    import concourse.tile as tile
    from concourse import mybir
    from contextlib import ExitStack

    T = C * nch
    FR = mybir.dt.float32r
    F32 = mybir.dt.float32
    BF = mybir.dt.bfloat16 if MLP_BF16 else FR
    AF = mybir.ActivationFunctionType

    nc = bacc.Bacc("TRN2", target_bir_lowering=False, debug=False,
                   num_devices=n_cores)

    din = {}
    def dt_in(name, shape, dt=FR):
        din[name] = nc.dram_tensor(name, list(shape), dt, kind="ExternalInput")
        return din[name]

    xc = dt_in("xc", [DC, T], BF)
    xt = dt_in("xt", [DT, T], BF)
    xcf = dt_in("xcf", [64, T])
    xid = dt_in("xid", [64, T])
    cw1 = dt_in("cw1", [DC, DC], BF)
    cw2 = dt_in("cw2", [DC, 256], BF)
    tw1 = dt_in("tw1", [DT, DT], BF)
    tw2 = dt_in("tw2", [DT, 256], BF)
    w3p = dt_in("w3p", [128, 4, 128])
    g22 = dt_in("g22", [128, 128])
    wv2 = dt_in("wv2", [128, 64])
    b1c = dt_in("b1c", [KC, 128], F32)
    b2c = dt_in("b2c", [2, 128], F32)
    b1t = dt_in("b1t", [KT_, 128], F32)
    b2t = dt_in("b2t", [2, 128], F32)
    b3 = dt_in("b3", [128, 1], F32)
    hotabw = dt_in("hotabw", [128, 144])
    abrepl = dt_in("abrepl", [36, 128])
    pickq = dt_in("pickq", [128, 256])
    hotq = dt_in("hotq", [128, 128])
    sq16 = dt_in("sq16", [16, 4])
    rep16 = dt_in("rep16", [4, 16])
    nsq = dt_in("nsq", [16, 40])
    pickr = dt_in("pickr", [128, 256])
    out = nc.dram_tensor("out", [64, T], F32, kind="ExternalOutput")

    xc_r = xc.rearrange("(kt p) t -> p kt t", p=128)
    xt_r = xt.rearrange("(kt p) t -> p kt t", p=128)

    with nc.allow_low_precision("float32r tiles feed float32r matmuls by design"), \
            tile.TileContext(nc) as tc:
        with ExitStack() as ctx:
            wp = ctx.enter_context(tc.tile_pool(name="wp", bufs=1))
            xin = ctx.enter_context(tc.tile_pool(name="xin", bufs=2))
            h1p = ctx.enter_context(tc.tile_pool(name="h1p", bufs=1))
            h2p = ctx.enter_context(tc.tile_pool(name="h2p", bufs=1))
            sqp = ctx.enter_context(tc.tile_pool(name="sqp", bufs=2))
            tmpp = ctx.enter_context(tc.tile_pool(name="tmpp", bufs=2))
            embp = ctx.enter_context(tc.tile_pool(name="embp", bufs=2))
            xnp = ctx.enter_context(tc.tile_pool(name="xnp", bufs=2))
            tinyp = ctx.enter_context(tc.tile_pool(name="tinyp", bufs=4))
            outp = ctx.enter_context(tc.tile_pool(name="outp", bufs=2))
            pbig = ctx.enter_context(tc.tile_pool(name="pbig", bufs=5,
                                                  space="PSUM"))
            psmall = ctx.enter_context(tc.tile_pool(name="psmall", bufs=1,
                                                    space="PSUM"))

            # ---- resident weights / consts ----
            cw1s = wp.tile([128, KC, DC], BF)
            nc.sync.dma_start(out=cw1s, in_=cw1.rearrange("(kt p) m -> p kt m", p=128))
            cw2s = wp.tile([128, KC, 256], BF)
            nc.sync.dma_start(out=cw2s, in_=cw2.rearrange("(kt p) m -> p kt m", p=128))
            tw1s = wp.tile([128, KT_, DT], BF)
            nc.sync.dma_start(out=tw1s, in_=tw1.rearrange("(kt p) m -> p kt m", p=128))
            tw2s = wp.tile([128, KT_, 256], BF)
            nc.sync.dma_start(out=tw2s, in_=tw2.rearrange("(kt p) m -> p kt m", p=128))
            w3ps = wp.tile([128, 4, 128], FR)
            nc.sync.dma_start(out=w3ps, in_=w3p[:, :, :])
            g22s = wp.tile([128, 128], FR)
            nc.sync.dma_start(out=g22s, in_=g22[:, :])
            wv2s = wp.tile([128, 64], FR)
            nc.sync.dma_start(out=wv2s, in_=wv2[:, :])
            b1cs = wp.tile([128, KC], F32)
            nc.sync.dma_start(out=b1cs, in_=b1c.rearrange("m p -> p m"))
            b2cs = wp.tile([128, 2], F32)
            nc.sync.dma_start(out=b2cs, in_=b2c.rearrange("m p -> p m"))
            b1ts = wp.tile([128, KT_], F32)
            nc.sync.dma_start(out=b1ts, in_=b1t.rearrange("m p -> p m"))
            b2ts = wp.tile([128, 2], F32)
            nc.sync.dma_start(out=b2ts, in_=b2t.rearrange("m p -> p m"))
            b3s = wp.tile([128, 1], F32)
            nc.sync.dma_start(out=b3s, in_=b3[:, :])
            hotabws = wp.tile([128, 144], FR)
            nc.sync.dma_start(out=hotabws, in_=hotabw[:, :])
            abrepls = wp.tile([36, 128], FR)
            nc.sync.dma_start(out=abrepls, in_=abrepl[:, :])
            pickqs = wp.tile([128, 256], FR)
            nc.sync.dma_start(out=pickqs, in_=pickq[:, :])
            hotqs = wp.tile([128, 128], FR)
            nc.sync.dma_start(out=hotqs, in_=hotq[:, :])
            sq16s = wp.tile([16, 4], FR)
            nc.sync.dma_start(out=sq16s, in_=sq16[:, :])
            rep16s = wp.tile([4, 16], FR)
            nc.sync.dma_start(out=rep16s, in_=rep16[:, :])
            nsqs = wp.tile([16, 40], FR)
            nc.sync.dma_start(out=nsqs, in_=nsq[:, :])
            pickrs = wp.tile([128, 256], FR)
            nc.sync.dma_start(out=pickrs, in_=pickr[:, :])

            from concourse.alu_op_type import AluOpType as ALU

            state = {}

            def make_units(j):
                """PE-dense MLP work for chunk j, as a list of emit fns."""
                sl = slice(j * C, (j + 1) * C)
                ctx_j = {}

                def u_load():
                    xc_j = xin.tile([128, KC, C], BF, tag="xc")
                    nc.sync.dma_start(out=xc_j, in_=xc_r[:, :, sl])
                    xt_j = xin.tile([128, KT_, C], BF, tag="xt")
                    nc.sync.dma_start(out=xt_j, in_=xt_r[:, :, sl])
                    emb_cfid = embp.tile([128, C], FR, tag="emb_cfid")
                    nc.sync.dma_start(out=emb_cfid[0:64, :], in_=xcf[:, sl])
                    nc.sync.dma_start(out=emb_cfid[64:128, :], in_=xid[:, sl])
                    ctx_j.update(xc_j=xc_j, xt_j=xt_j, emb_cfid=emb_cfid)

                def l1_tile(xj_key, kt, w1s, b1sT, tag, m):
                    def emit():
                        xj = ctx_j[xj_key]
                        if m == 0:
                            h1 = h1p.tile([128, kt, C], BF, tag="h1" + tag)
                            ctx_j["h1" + tag] = h1
                        h1 = ctx_j["h1" + tag]
                        ps = pbig.tile([128, C], F32, tag="mm", name="ps")
                        for k in range(kt):
                            nc.tensor.matmul(
                                ps[:, :], w1s[:, k, 128 * m:128 * (m + 1)],
                                xj[:, k, :], start=(k == 0), stop=(k == kt - 1))
                        nc.scalar.activation(out=h1[:, m, :], in_=ps[:, :],
                                             func=AF.Relu,
                                             bias=b1sT[:, m:m + 1])
                    return emit

                def l2_tile(kt, w2s, b2sT, tag, m):
                    def emit():
                        h1 = ctx_j["h1" + tag]
                        if m == 0:
                            h2 = h2p.tile([128, 2, C], FR, tag="h2" + tag)
                            ctx_j["h2" + tag] = h2
                        h2 = ctx_j["h2" + tag]
                        ps = pbig.tile([128, C], F32, tag="mm")
                        for k in range(kt):
                            nc.tensor.matmul(
                                ps[:, :], w2s[:, k, 128 * m:128 * (m + 1)],
                                h1[:, k, :], start=(k == 0), stop=(k == kt - 1))
                        nc.scalar.activation(out=h2[:, m, :], in_=ps[:, :],
                                             func=AF.Relu,
                                             bias=b2sT[:, m:m + 1])
                    return emit

                def u_l3cf():
                    h2c, h2t = ctx_j["h2c"], ctx_j["h2t"]
                    ps3 = pbig.tile([128, C], F32, tag="mm")
                    nc.tensor.matmul(ps3[:, :], w3ps[:, 0, :], h2c[:, 0, :],
                                     start=True, stop=False)
                    nc.tensor.matmul(ps3[:, :], w3ps[:, 1, :], h2c[:, 1, :],
                                     start=False, stop=False)
                    nc.tensor.matmul(ps3[:, :], w3ps[:, 2, :], h2t[:, 0, :],
                                     start=False, stop=False)
                    nc.tensor.matmul(ps3[:, :], w3ps[:, 3, :], h2t[:, 1, :],
                                     start=False, stop=True)
                    emb_ct = embp.tile([128, C], FR, tag="emb_ct")
                    nc.scalar.activation(out=emb_ct, in_=ps3[:, :],
                                         func=AF.Identity, bias=b3s[:, :])
                    state[j] = (emb_ct, ctx_j["emb_cfid"])

                units = [u_load]
                units += [l1_tile("xc_j", KC, cw1s, b1cs, "c", m)
                          for m in range(KC)]
                units += [l2_tile(KC, cw2s, b2cs, "c", m) for m in range(2)]
                units += [l1_tile("xt_j", KT_, tw1s, b1ts, "t", m)
                          for m in range(KT_)]
                units += [l2_tile(KT_, tw2s, b2ts, "t", m) for m in range(2)]
                units += [u_l3cf]
                return units

            def make_stages(j):
                """attention for chunk j (embs from state[j]), as emit fns."""
                sl = slice(j * C, (j + 1) * C)
                actx = {}

                def s_stats():
                    emb_ct, emb_cfid = state.pop(j)
                    actx["emb_ct"], actx["emb_cfid"] = emb_ct, emb_cfid
                    sq_ct = sqp.tile([128, C], FR, tag="sqs")
                    nc.scalar.activation(out=sq_ct, in_=emb_ct, func=AF.Square)
                    sq_cfid = sqp.tile([128, C], FR, tag="sqs")
                    nc.scalar.activation(out=sq_cfid, in_=emb_cfid,
                                         func=AF.Square)
                    st36 = psmall.tile([36, C], F32, tag="st", bufs=1,
                                       name="st36")
                    nc.tensor.matmul(st36[0:36, :], hotabws[:, 0:36], emb_ct,
                                     start=True, stop=False)
                    nc.tensor.matmul(st36[0:36, :], hotabws[:, 36:72], emb_cfid,
                                     start=False, stop=False)
                    nc.tensor.matmul(st36[0:36, :], hotabws[:, 72:108], sq_ct,
                                     start=False, stop=False)
                    nc.tensor.matmul(st36[0:36, :], hotabws[:, 108:144], sq_cfid,
                                     start=False, stop=True)
                    actx["st36"] = st36

                def s_ab():
                    # folded l2norm+LN: A = rsqrt(ss*(1/64+eps) - mu^2),
                    # B = mu*A; then replicate to rows {0,32,64,96}+4 via PE.
                    st36 = actx.pop("st36")
                    mu4 = tinyp.tile([4, C], FR, tag="t4c")
                    nc.vector.tensor_scalar_mul(mu4, st36[0:4, :], 1.0 / 64)
                    musq = tinyp.tile([4, C], FR, tag="t4c")
                    nc.vector.tensor_mul(musq, mu4, mu4)
                    apre = tinyp.tile([4, C], FR, tag="t4c")
                    nc.vector.scalar_tensor_tensor(
                        apre, st36[32:36, :], 1.0 / 64 + LN_EPS, musq,
                        op0=ALU.mult, op1=ALU.subtract)
                    asq = tinyp.tile([4, C], FR, tag="t4c")
                    nc.scalar.activation(out=asq, in_=apre, func=AF.Sqrt)
                    ab36 = tmpp.tile([36, C], FR, tag="ab36", bufs=2)
                    nc.vector.reciprocal(ab36[0:4, :], asq)
                    nc.vector.tensor_mul(ab36[32:36, :], mu4, ab36[0:4, :])
                    abp = pbig.tile([128, C], F32, tag="mm", name="abp")
                    nc.tensor.matmul(abp[:, :], abrepls[:, :], ab36,
                                     start=True, stop=True)
                    ab4 = tmpp.tile([128, C], FR, tag="ab4", bufs=2)
                    nc.vector.tensor_copy(ab4, abp[:, :])
                    actx["ab4"] = ab4

                def s_xn():
                    # 4 concurrent row-tiled K=4 broadcasts -> A/B [128, C]
                    ab4 = actx.pop("ab4")
                    bps = []
                    for i, (r0, cs) in enumerate([(0, slice(0, 128)),
                                                  (32, slice(0, 128)),
                                                  (64, slice(128, 256)),
                                                  (96, slice(128, 256))]):
                        bp = pbig.tile([128, C], F32, tag="mm", name="bp")
                        nc.tensor.matmul(bp[:, :], pickqs[r0:r0 + 4, cs],
                                         ab4[r0:r0 + 4, :],
                                         start=True, stop=True,
                                         tile_position=(r0, 0))
                        bps.append(bp)
                    for pk, (ekey, ia, ib, tag) in enumerate(
                            [("emb_ct", 0, 1, "ct"), ("emb_cfid", 2, 3, "cfid")]):
                        tmp2 = tmpp.tile([128, C], FR, tag="tmp")
                        nc.vector.tensor_mul(tmp2, actx[ekey], bps[ia][:, :])
                        xn = xnp.tile([128, C], FR, tag="xn" + tag, name="xn")
                        nc.vector.tensor_sub(xn, tmp2, bps[ib][:, :])
                        actx["xn" + tag] = xn

                def s_qg(p):
                    def emit():
                        xn_p = actx["xnct" if p == 0 else "xncfid"]
                        qt = pbig.tile([128, C], F32, tag="mm", name="qt")
                        nc.tensor.matmul(qt[:, :], g22s[0:64, :],
                                         xn_p[0:64, :], start=True, stop=True,
                                         tile_position=(0, 0))
                        qb = pbig.tile([128, C], F32, tag="mm", name="qb")
                        nc.tensor.matmul(qb[:, :], g22s[64:128, :],
                                         xn_p[64:128, :], start=True,
                                         stop=True, tile_position=(64, 0))
                        actx["qg"] = (qt, qb)
                    return emit

                def s_pr(p):
                    def emit():
                        qt, qb = actx.pop("qg")
                        prs = []
                        for hi, qg_t in enumerate((qt, qb)):
                            for nk in ("xnct", "xncfid"):
                                pr = tmpp.tile([128, C], FR, tag="pr",
                                               name="pr", bufs=4)
                                nc.vector.tensor_mul(pr, qg_t[:, :], actx[nk])
                                prs.append(pr)
                        actx["prs"] = prs
                    return emit

                def s_dots(p):
                    def emit():
                        prs = actx.pop("prs")
                        if p == 0:
                            actx["s16"] = psmall.tile([16, C], F32, tag="s16",
                                                      bufs=1, name="s16")
                        s16 = actx["s16"]
                        for i, pr in enumerate(prs):
                            b = 4 * p + i
                            nc.tensor.matmul(s16[:, :],
                                             hotqs[:, 16 * b:16 * (b + 1)],
                                             pr, start=(p == 0 and i == 0),
                                             stop=(p == 1 and i == 3))
                    return emit

                def s_soft1():
                    s16 = actx.pop("s16")
                    e16 = tmpp.tile([16, C], FR, tag="e16", bufs=2)
                    nc.scalar.activation(out=e16, in_=s16[:, :], func=AF.Exp)
                    soft = psmall.tile([40, C], F32, tag="soft", bufs=1,
                                       name="soft")
                    nc.tensor.matmul(soft[0:4, :], sq16s[:, :], e16,
                                     start=True, stop=True)
                    rr4 = tinyp.tile([4, C], FR, tag="t4c")
                    nc.vector.reciprocal(rr4, soft[0:4, :])
                    actx["e16"], actx["rr4"] = e16, rr4

                def s_soft2():
                    rrep = psmall.tile([40, C], F32, tag="soft", bufs=1,
                                       name="rrep")
                    nc.tensor.matmul(rrep[0:16, :], rep16s[:, :],
                                     actx["rr4"], start=True, stop=True)
                    w16 = tmpp.tile([16, C], FR, tag="w16", bufs=2)
                    nc.vector.tensor_mul(w16, actx.pop("e16"), rrep[0:16, :])
                    actx["w16"] = w16

                def s_out1():
                    a440 = psmall.tile([40, C], F32, tag="soft", bufs=1,
                                       name="a440")
                    nc.tensor.matmul(a440[:, :], nsqs[:, :], actx.pop("w16"),
                                     start=True, stop=True)
                    a4s = tmpp.tile([40, C], FR, tag="a4s", bufs=2)
                    nc.vector.tensor_copy(a4s, a440[:, :])
                    actx["a4s"] = a4s

                def s_out2():
                    a4s = actx.pop("a4s")
                    arp_ct = pbig.tile([128, C], F32, tag="mm", name="arp_ct")
                    nc.tensor.matmul(arp_ct[:, :], pickrs[0:4, 0:128],
                                     a4s[0:4, :], start=True, stop=True,
                                     tile_position=(0, 0))
                    arp_cf = pbig.tile([128, C], F32, tag="mm", name="arp_cf")
                    nc.tensor.matmul(arp_cf[:, :], pickrs[32:36, 128:256],
                                     a4s[32:36, :], start=True, stop=True,
                                     tile_position=(32, 0))
                    zp_ct = tmpp.tile([128, C], FR, tag="pr", name="zp_ct",
                                      bufs=4)
                    nc.vector.tensor_mul(zp_ct, actx["xnct"], arp_ct[:, :])
                    zp_cf = tmpp.tile([128, C], FR, tag="pr", name="zp_cf",
                                      bufs=4)
                    nc.vector.tensor_mul(zp_cf, actx["xncfid"], arp_cf[:, :])
                    zsum = tmpp.tile([128, C], FR, tag="tmp", name="zsum")
                    nc.vector.tensor_add(zsum, zp_ct, zp_cf)
                    actx["zsum"] = zsum

                def s_out3():
                    fps = pbig.tile([128, C], F32, tag="mm")
                    nc.tensor.matmul(fps[0:64, :], wv2s[:, :],
                                     actx.pop("zsum"), start=True, stop=True)
                    out_sb = outp.tile([64, C], F32, tag="osb")
                    nc.vector.tensor_copy(out_sb, fps[0:64, :])
                    nc.sync.dma_start(out=out[:, sl], in_=out_sb)

                return [s_stats, s_ab, s_xn,
                        s_qg(0), s_pr(0), s_dots(0),
                        s_qg(1), s_pr(1), s_dots(1),
                        s_soft1, s_soft2, s_out1, s_out2, s_out3]

            import os
            skip_attn = bool(int(os.environ.get("K_SKIP_ATTN", "0")))
            skip_mlp = bool(int(os.environ.get("K_SKIP_MLP", "0")))

            def emit_all():
                for it in range(nch + 1):
                    units = make_units(it) if it < nch else []
                    stages = (make_stages(it - 1) if it >= 1 else [])
                    if skip_attn:
                        stages = []
                        if it >= 1:
                            state.pop(it - 1, None)
                    if skip_mlp:
                        units = units[:1]  # loads only
                        stages = []
                    # weave: distribute stages evenly among units
                    n_u, n_s = len(units), len(stages)
                    si = 0
                    for ui, u in enumerate(units):
                        u()
                        want = ((ui + 1) * n_s) // max(n_u, 1)
                        while si < want:
                            stages[si]()
                            si += 1
                    while si < n_s:
                        stages[si]()
                        si += 1

            if repeat == 1:
                emit_all()
            else:
                with tc.For_i(0, repeat, 1):
                    emit_all()

    nc.finalize()
    return nc


def _get_nc(nch=NCH0):
    key = ("nc", nch)
    if key not in _CACHE:
        _CACHE[key] = _build_nc(nch=nch)
    return _CACHE[key]


def _l2norm_rows(x):
    n = np.sqrt(np.einsum("ij,ij->i", x, x, dtype=np.float64))
    n = np.maximum(n, 1e-12).astype(np.float32)
    return x / n[:, None]


def _host_prep(inputs, nch=None):
    """Dedup + pre-gather + normalize.  Returns (in_maps, inv, nch)."""
    f32 = np.float32
    seq = np.asarray(inputs["seq_modify"])
    seq = np.where(seq == NUM_ITEM, 0, seq).astype(np.int64)  # [1024, 50]
    uniq, inv = np.unique(seq.ravel(), return_inverse=True)
    if nch is None:
        nch = max(1, -(-uniq.size // (N_CORES * C)))
    T = C * nch
    total = N_CORES * T
    assert uniq.size <= total
    idx_full = np.zeros(total, np.int64)
    idx_full[:uniq.size] = uniq

    cf_full = _l2norm_rows(np.asarray(inputs["content_feature"], dtype=f32))
    tf_full = _l2norm_rows(np.asarray(inputs["text_feature"], dtype=f32))
    cff_full = np.asarray(inputs["cf_feature"], dtype=f32)
    ide_full = np.asarray(inputs["item_embeddings"], dtype=f32)

    c_w3 = np.asarray(inputs["c_w3"], dtype=f32)   # [64, 256]
    t_w3 = np.asarray(inputs["t_w3"], dtype=f32)
    cw3T = np.ascontiguousarray(c_w3.T)            # [256, 64]
    tw3T = np.ascontiguousarray(t_w3.T)
    w3p = np.zeros((128, 4, 128), f32)
    w3p[:, 0, 0:64] = cw3T[0:128]
    w3p[:, 1, 0:64] = cw3T[128:256]
    w3p[:, 2, 64:128] = tw3T[0:128]
    w3p[:, 3, 64:128] = tw3T[128:256]
    cf_w = np.asarray(inputs["cf_w"], dtype=f32)
    cfe_full = (cff_full @ cf_w.T
                + np.asarray(inputs["cf_b"], dtype=f32)[None, :])
    wq = np.asarray(inputs["wq"], dtype=f32)
    wk = np.asarray(inputs["wk"], dtype=f32)
    wv = np.asarray(inputs["wv"], dtype=f32)
    G = (wq.T @ wk) * (D ** -0.5)
    G2 = np.concatenate([G, G], axis=1)            # [64, 128]
    g22 = np.concatenate([G2, G2], axis=0)         # [128, 128]
    wv2 = np.concatenate([wv.T, wv.T], axis=0)     # [128, 64]
    b3 = np.concatenate([np.asarray(inputs["c_b3"], dtype=f32),
                         np.asarray(inputs["t_b3"], dtype=f32)])[:, None]

    shared = dict(
        cw1=np.ascontiguousarray(np.asarray(inputs["c_w1"], dtype=f32).T
                                 .astype(_bf())),
        cw2=np.ascontiguousarray(np.asarray(inputs["c_w2"], dtype=f32).T
                                 .astype(_bf())),
        tw1=np.ascontiguousarray(np.asarray(inputs["t_w1"], dtype=f32).T
                                 .astype(_bf())),
        tw2=np.ascontiguousarray(np.asarray(inputs["t_w2"], dtype=f32).T
                                 .astype(_bf())),
        w3p=w3p, g22=g22, wv2=wv2,
        b1c=np.asarray(inputs["c_b1"], dtype=f32).reshape(KC, 128),
        b2c=np.asarray(inputs["c_b2"], dtype=f32).reshape(2, 128),
        b1t=np.asarray(inputs["t_b1"], dtype=f32).reshape(KT_, 128),
        b2t=np.asarray(inputs["t_b2"], dtype=f32).reshape(2, 128),
        b3=b3,
        **_build_consts(),
    )

    in_maps = []
    for c in range(N_CORES):
        idx = idx_full[c * T:(c + 1) * T]
        m = dict(shared)
        m["xc"] = np.ascontiguousarray(cf_full[idx].T.astype(_bf()))
        m["xt"] = np.ascontiguousarray(tf_full[idx].T.astype(_bf()))
        m["xcf"] = np.ascontiguousarray(cfe_full[idx].T)
        m["xid"] = np.ascontiguousarray(ide_full[idx].T)
        in_maps.append(m)
    return in_maps, inv, nch


def _get_runner(nc=None, key="runner", nch=NCH0):
    """Cached jitted shard_map runner over 8 cores (mirrors
    bass2jax.run_bass_via_pjrt but reuses one jit so repeat calls skip
    retracing)."""
    if key in _CACHE:
        return _CACHE[key]
    import jax
    from jax.sharding import Mesh, PartitionSpec
    try:
        from jax.experimental.shard_map import shard_map
    except ImportError:
        from jax.shard_map import shard_map
    from concourse import bass2jax, mybir

    if nc is None:
        nc = _get_nc(nch)
    bass2jax.install_neuronx_cc_hook()
    partition_name = (nc.partition_id_tensor.name
                      if nc.partition_id_tensor else None)
    in_names, out_names, out_avals, zero_shapes = [], [], [], []
    for alloc in nc.m.functions[0].allocations:
        if not isinstance(alloc, mybir.MemoryLocationSet):
            continue
        name = alloc.memorylocations[0].name
        if alloc.kind == "ExternalInput":
            if name != partition_name:
                in_names.append(name)
        elif alloc.kind == "ExternalOutput":
            out_names.append(name)
            shape = tuple(alloc.tensor_shape)
            dtype = mybir.dt.np(alloc.dtype)
            out_avals.append(jax.core.ShapedArray(shape, dtype))
            zero_shapes.append((shape, dtype))
    n_params = len(in_names)
    full_in_names = list(in_names) + list(out_names)
    if partition_name is not None:
        full_in_names.append(partition_name)

    def _body(*args):
        operands = list(args)
        if partition_name is not None:
            operands.append(bass2jax.partition_id_tensor())
        outs = bass2jax._bass_exec_p.bind(
            *operands,
            out_avals=tuple(out_avals),
            in_names=tuple(full_in_names),
            out_names=tuple(out_names),
            lowering_input_output_aliases=(),
            sim_require_finite=True,
            sim_require_nnan=True,
            nc=nc,
        )
        return tuple(outs)

    devices = jax.devices()[:N_CORES]
    mesh = Mesh(np.asarray(devices), ("core",))
    n_outs = len(out_names)
    in_specs = (PartitionSpec("core"),) * (n_params + n_outs)
    out_specs = (PartitionSpec("core"),) * n_outs
    sharded = jax.jit(
        shard_map(_body, mesh=mesh, in_specs=in_specs, out_specs=out_specs,
                  check_rep=False),
        keep_unused=True,
    )
    runner = (sharded, in_names, out_names, zero_shapes, mesh)
    _CACHE[key] = runner
    return runner


def _run_device(in_maps, nch):
    sharded, in_names, out_names, zero_shapes, _ = _get_runner(
        key=("runner", nch), nch=nch)
    concat_in = [
        np.concatenate([np.asarray(in_maps[c][n]) for c in range(N_CORES)],
                       axis=0)
        for n in in_names
    ]
    concat_zeros = [np.zeros((N_CORES * s[0], *s[1:]), d)
                    for (s, d) in zero_shapes]
    out_arrs = sharded(*concat_in, *concat_zeros)
    return np.asarray(out_arrs[out_names.index("out")])


def kernel(**inputs):
    in_maps, inv, nch = _host_prep(inputs)
    out_cat = _run_device(in_maps, nch)     # [8*64, T]
    T = C * nch
    f_all = (out_cat.reshape(N_CORES, 64, T)
             .transpose(0, 2, 1).reshape(N_CORES * T, 64))
    return f_all[inv].reshape(B, S, D).astype(np.float32)


# revision 17
# speedup vs baseline: 1.0115x; 1.0115x over previous
"""Trainium2 Bass kernel for nn_DisentangledHierarchicalEncoder.

Strategy (8 NeuronCores, SPMD, zero collectives):
  Every token's output depends only on its item id (the 4-modal attention is
  within-token), so the host dedups seq_modify to unique items (~32k of 50k),
  shards those across the 8 cores, pre-gathers + L2-normalizes the raw
  features on the host (indices are host-known), and scatters the per-item
  outputs back to tokens afterwards.  Each core runs a dense pipeline over
  its ~4096 item slots in chunks of 512:
      content MLP (1024->1024->256->64), text MLP (768->768->256->64),
      cf linear (64->64), id passthrough,
      per-(item, modality) l2norm + LayerNorm folded into one affine,
      4x4 self-attention (scores via G = 0.125 * wq.T @ wk), mean-pool.
  L1/L2 matmuls run in bf16, the rest in float32r.  Per-item scalars are
  broadcast across partitions with PE outer-products against host-supplied
  pick matrices; partition-stacked pairs (content|text and cf|id) keep every
  engine op on 128 partitions.
"""

import numpy as np

NUM_ITEM = 50000
B, S, D = 1024, 50, 64
DC, DT = 1024, 768
N_CORES = 8
C = 512                           # chunk width (item slots per chunk)
NCH0 = 8                          # default chunks per core (covers ~32.7k uniq)
KC, KT_ = DC // 128, DT // 128    # k-tiles: 8 content, 6 text
LN_EPS = 1e-5
MLP_BF16 = True                   # L1/L2 matmuls in bf16, rest float32r

_CACHE = {}


def _bf():
    if not MLP_BF16:
        return np.float32
    import ml_dtypes
    return ml_dtypes.bfloat16


def _build_consts():
    """Packed attention lhsT constants (modal order 0=c 1=t 2=cf 3=id;
    score row r = 4*m + n)."""
    f32 = np.float32
    # stats: st36 rows 0:4 sums, 32:36 sum-of-squares (32-aligned reads).
    # 4 slices of 36 cols for emb_ct / emb_cfid / sq_ct / sq_cfid.
    hotabw = np.zeros((128, 144), f32)
    for si, (ct, cb) in enumerate([(0, 1), (2, 3), (32, 33), (34, 35)]):
        hotabw[0:64, 36 * si + ct] = 1.0
        hotabw[64:128, 36 * si + cb] = 1.0
    # AB36 [A4 @ 0:4; B4 @ 32:36] -> AB4 rows {0:4 A, 32:36 B, 64:68 A, 96:100 B}
    abrepl = np.zeros((36, 128), f32)
    for j in range(4):
        abrepl[j, j] = 1.0
        abrepl[32 + j, 32 + j] = 1.0
        abrepl[j, 64 + j] = 1.0
        abrepl[32 + j, 96 + j] = 1.0
    # pick: [4, 256]; cols 0:128 broadcast rows 0/1 to halves, cols 128:256 rows 2/3
    pickp = np.zeros((4, 256), f32)
    pickp[0, 0:64] = 1.0
    pickp[1, 64:128] = 1.0
    pickp[2, 128 + 0:128 + 64] = 1.0
    pickp[3, 128 + 64:128 + 128] = 1.0
    # xn broadcasts, row-tiled at 0/32/64/96: (A,ct) (B,ct) (A,cfid) (B,cfid)
    pickq = np.zeros((128, 256), f32)
    pickq[0:4, 0:128] = pickp[:, 0:128]
    pickq[32:36, 0:128] = pickp[:, 0:128]
    pickq[64:68, 128:256] = pickp[:, 128:256]
    pickq[96:100, 128:256] = pickp[:, 128:256]
    # score dots: block b = 2*mu + q (q: 0=ct-pack 1=cfid-pack); top half ->
    # row 4*mu + nu_top(q), bottom -> 4*mu + nu_bot(q).
    nu = {0: (0, 1), 1: (2, 3)}
    hotq = np.zeros((128, 128), f32)
    for mu_i in range(4):
        for q in range(2):
            b = 2 * mu_i + q
            hotq[0:64, 16 * b + 4 * mu_i + nu[q][0]] = 1.0
            hotq[64:128, 16 * b + 4 * mu_i + nu[q][1]] = 1.0
    # softmax sums over n: se row m = sum_n e16 row 4m+n
    sq16 = np.zeros((16, 4), f32)
    for m in range(4):
        for n in range(4):
            sq16[4 * m + n, m] = 1.0
    # replicate rr rows m -> rows 4m+n
    rep16 = np.zeros((4, 16), f32)
    for m in range(4):
        for n in range(4):
            rep16[m, 4 * m + n] = 1.0
    # a4[n] = sum_m w16 row 4m+n, written at rows {n, 32+n}
    nsq = np.zeros((16, 40), f32)
    for m in range(4):
        for n in range(4):
            nsq[4 * m + n, n] = 1.0
            nsq[4 * m + n, 32 + n] = 1.0
    # arp broadcasts, row-tiled at 0/32: ct-pack from a4 rows 0:4,
    # cfid-pack from a4 rows 32:36
    pickr = np.zeros((128, 256), f32)
    pickr[0:4, 0:128] = pickp[:, 0:128]
    pickr[32:36, 128:256] = pickp[:, 128:256]
    return dict(hotabw=hotabw, abrepl=abrepl, pickq=pickq, hotq=hotq,
                sq16=sq16, rep16=rep16, nsq=nsq, pickr=pickr)


def _build_nc(nch=NCH0, n_cores=N_CORES, repeat=1):
    import concourse.bacc as bacc
    import concourse.tile as tile
    from concourse import mybir
    from contextlib import ExitStack

    T = C * nch
    FR = mybir.dt.float32r
    F32 = mybir.dt.float32
    BF = mybir.dt.bfloat16 if MLP_BF16 else FR
    AF = mybir.ActivationFunctionType

    nc = bacc.Bacc("TRN2", target_bir_lowering=False, debug=False,
                   num_devices=n_cores)

    din = {}
    def dt_in(name, shape, dt=FR):
        din[name] = nc.dram_tensor(name, list(shape), dt, kind="ExternalInput")
        return din[name]

    xc = dt_in("xc", [DC, T], BF)
    xt = dt_in("xt", [DT, T], BF)
    xcf = dt_in("xcf", [64, T])
    xid = dt_in("xid", [64, T])
    cw1 = dt_in("cw1", [DC, DC], BF)
    cw2 = dt_in("cw2", [DC, 256], BF)
    tw1 = dt_in("tw1", [DT, DT], BF)
    tw2 = dt_in("tw2", [DT, 256], BF)
    w3p = dt_in("w3p", [128, 4, 128])
    cfwp = dt_in("cfwp", [64, 128])
    g22 = dt_in("g22", [128, 128])
    wv2 = dt_in("wv2", [128, 64])
    b1c = dt_in("b1c", [KC, 128], F32)
    b2c = dt_in("b2c", [2, 128], F32)
    b1t = dt_in("b1t", [KT_, 128], F32)
    b2t = dt_in("b2t", [2, 128], F32)
    b3 = dt_in("b3", [128, 1], F32)
    bcf = dt_in("bcf", [64, 1], F32)
    hotabw = dt_in("hotabw", [128, 144])
    abrepl = dt_in("abrepl", [36, 128])
    pickq = dt_in("pickq", [128, 256])
    hotq = dt_in("hotq", [128, 128])
    sq16 = dt_in("sq16", [16, 4])
    rep16 = dt_in("rep16", [4, 16])
    nsq = dt_in("nsq", [16, 40])
    pickr = dt_in("pickr", [128, 256])
    out = nc.dram_tensor("out", [64, T], F32, kind="ExternalOutput")

    xc_r = xc.rearrange("(kt p) t -> p kt t", p=128)
    xt_r = xt.rearrange("(kt p) t -> p kt t", p=128)

    with nc.allow_low_precision("float32r tiles feed float32r matmuls by design"), \
            tile.TileContext(nc) as tc:
        with ExitStack() as ctx:
            wp = ctx.enter_context(tc.tile_pool(name="wp", bufs=1))
            xin = ctx.enter_context(tc.tile_pool(name="xin", bufs=2))
            h1p = ctx.enter_context(tc.tile_pool(name="h1p", bufs=1))
            h2p = ctx.enter_context(tc.tile_pool(name="h2p", bufs=1))
            sqp = ctx.enter_context(tc.tile_pool(name="sqp", bufs=2))
            tmpp = ctx.enter_context(tc.tile_pool(name="tmpp", bufs=2))
            embp = ctx.enter_context(tc.tile_pool(name="embp", bufs=2))
            xnp = ctx.enter_context(tc.tile_pool(name="xnp", bufs=2))
            tinyp = ctx.enter_context(tc.tile_pool(name="tinyp", bufs=4))
            outp = ctx.enter_context(tc.tile_pool(name="outp", bufs=2))
            pbig = ctx.enter_context(tc.tile_pool(name="pbig", bufs=5,
                                                  space="PSUM"))
            psmall = ctx.enter_context(tc.tile_pool(name="psmall", bufs=1,
                                                    space="PSUM"))

            # ---- resident weights / consts ----
            cw1s = wp.tile([128, KC, DC], BF)
            nc.sync.dma_start(out=cw1s, in_=cw1.rearrange("(kt p) m -> p kt m", p=128))
            cw2s = wp.tile([128, KC, 256], BF)
            nc.sync.dma_start(out=cw2s, in_=cw2.rearrange("(kt p) m -> p kt m", p=128))
            tw1s = wp.tile([128, KT_, DT], BF)
            nc.sync.dma_start(out=tw1s, in_=tw1.rearrange("(kt p) m -> p kt m", p=128))
            tw2s = wp.tile([128, KT_, 256], BF)
            nc.sync.dma_start(out=tw2s, in_=tw2.rearrange("(kt p) m -> p kt m", p=128))
            w3ps = wp.tile([128, 4, 128], FR)
            nc.sync.dma_start(out=w3ps, in_=w3p[:, :, :])
            cfwps = wp.tile([64, 128], FR)
            nc.sync.dma_start(out=cfwps, in_=cfwp[:, :])
            g22s = wp.tile([128, 128], FR)
            nc.sync.dma_start(out=g22s, in_=g22[:, :])
            wv2s = wp.tile([128, 64], FR)
            nc.sync.dma_start(out=wv2s, in_=wv2[:, :])
            b1cs = wp.tile([128, KC], F32)
            nc.sync.dma_start(out=b1cs, in_=b1c.rearrange("m p -> p m"))
            b2cs = wp.tile([128, 2], F32)
            nc.sync.dma_start(out=b2cs, in_=b2c.rearrange("m p -> p m"))
            b1ts = wp.tile([128, KT_], F32)
            nc.sync.dma_start(out=b1ts, in_=b1t.rearrange("m p -> p m"))
            b2ts = wp.tile([128, 2], F32)
            nc.sync.dma_start(out=b2ts, in_=b2t.rearrange("m p -> p m"))
            b3s = wp.tile([128, 1], F32)
            nc.sync.dma_start(out=b3s, in_=b3[:, :])
            bcfs = wp.tile([64, 1], F32)
            nc.sync.dma_start(out=bcfs, in_=bcf[:, :])
            hotabws = wp.tile([128, 144], FR)
            nc.sync.dma_start(out=hotabws, in_=hotabw[:, :])
            abrepls = wp.tile([36, 128], FR)
            nc.sync.dma_start(out=abrepls, in_=abrepl[:, :])
            pickqs = wp.tile([128, 256], FR)
            nc.sync.dma_start(out=pickqs, in_=pickq[:, :])
            hotqs = wp.tile([128, 128], FR)
            nc.sync.dma_start(out=hotqs, in_=hotq[:, :])
            sq16s = wp.tile([16, 4], FR)
            nc.sync.dma_start(out=sq16s, in_=sq16[:, :])
            rep16s = wp.tile([4, 16], FR)
            nc.sync.dma_start(out=rep16s, in_=rep16[:, :])
            nsqs = wp.tile([16, 40], FR)
            nc.sync.dma_start(out=nsqs, in_=nsq[:, :])
            pickrs = wp.tile([128, 256], FR)
            nc.sync.dma_start(out=pickrs, in_=pickr[:, :])

            from concourse.alu_op_type import AluOpType as ALU

            state = {}

            def make_units(j):
                """PE-dense MLP work for chunk j, as a list of emit fns."""
                sl = slice(j * C, (j + 1) * C)
                ctx_j = {}

                def u_load():
                    xc_j = xin.tile([128, KC, C], BF, tag="xc")
                    nc.sync.dma_start(out=xc_j, in_=xc_r[:, :, sl])
                    xt_j = xin.tile([128, KT_, C], BF, tag="xt")
                    nc.sync.dma_start(out=xt_j, in_=xt_r[:, :, sl])
                    xcf_j = xin.tile([64, C], FR, tag="xcf")
                    nc.sync.dma_start(out=xcf_j, in_=xcf[:, sl])
                    emb_cfid = embp.tile([128, C], FR, tag="emb_cfid")
                    nc.sync.dma_start(out=emb_cfid[64:128, :], in_=xid[:, sl])
                    ctx_j.update(xc_j=xc_j, xt_j=xt_j, xcf_j=xcf_j,
                                 emb_cfid=emb_cfid)

                def l1_tile(xj_key, kt, w1s, b1sT, tag, m):
                    def emit():
                        xj = ctx_j[xj_key]
                        if m == 0:
                            h1 = h1p.tile([128, kt, C], BF, tag="h1" + tag)
                            ctx_j["h1" + tag] = h1
                        h1 = ctx_j["h1" + tag]
                        ps = pbig.tile([128, C], F32, tag="mm", name="ps")
                        for k in range(kt):
                            nc.tensor.matmul(
                                ps[:, :], w1s[:, k, 128 * m:128 * (m + 1)],
                                xj[:, k, :], start=(k == 0), stop=(k == kt - 1))
                        nc.scalar.activation(out=h1[:, m, :], in_=ps[:, :],
                                             func=AF.Relu,
                                             bias=b1sT[:, m:m + 1])
                    return emit

                def l2_tile(kt, w2s, b2sT, tag, m):
                    def emit():
                        h1 = ctx_j["h1" + tag]
                        if m == 0:
                            h2 = h2p.tile([128, 2, C], FR, tag="h2" + tag)
                            ctx_j["h2" + tag] = h2
                        h2 = ctx_j["h2" + tag]
                        ps = pbig.tile([128, C], F32, tag="mm")
                        for k in range(kt):
                            nc.tensor.matmul(
                                ps[:, :], w2s[:, k, 128 * m:128 * (m + 1)],
                                h1[:, k, :], start=(k == 0), stop=(k == kt - 1))
                        nc.scalar.activation(out=h2[:, m, :], in_=ps[:, :],
                                             func=AF.Relu,
                                             bias=b2sT[:, m:m + 1])
                    return emit

                def u_l3cf():
                    h2c, h2t = ctx_j["h2c"], ctx_j["h2t"]
                    ps3 = pbig.tile([128, C], F32, tag="mm")
                    nc.tensor.matmul(ps3[:, :], w3ps[:, 0, :], h2c[:, 0, :],
                                     start=True, stop=False)
                    nc.tensor.matmul(ps3[:, :], w3ps[:, 1, :], h2c[:, 1, :],
                                     start=False, stop=False)
                    nc.tensor.matmul(ps3[:, :], w3ps[:, 2, :], h2t[:, 0, :],
                                     start=False, stop=False)
                    nc.tensor.matmul(ps3[:, :], w3ps[:, 3, :], h2t[:, 1, :],
                                     start=False, stop=True)
                    emb_ct = embp.tile([128, C], FR, tag="emb_ct")
                    nc.scalar.activation(out=emb_ct, in_=ps3[:, :],
                                         func=AF.Identity, bias=b3s[:, :])
                    pcf = pbig.tile([128, C], F32, tag="mm")
                    nc.tensor.matmul(pcf[:, :], cfwps[:, :], ctx_j["xcf_j"],
                                     start=True, stop=True)
                    emb_cfid = ctx_j["emb_cfid"]
                    nc.scalar.activation(out=emb_cfid[0:64, :],
                                         in_=pcf[0:64, :],
                                         func=AF.Identity, bias=bcfs[:, :])
                    state[j] = (emb_ct, emb_cfid)

                units = [u_load]
                units += [l1_tile("xc_j", KC, cw1s, b1cs, "c", m)
                          for m in range(KC)]
                units += [l2_tile(KC, cw2s, b2cs, "c", m) for m in range(2)]
                units += [l1_tile("xt_j", KT_, tw1s, b1ts, "t", m)
                          for m in range(KT_)]
                units += [l2_tile(KT_, tw2s, b2ts, "t", m) for m in range(2)]
                units += [u_l3cf]
                return units

            def make_stages(j):
                """attention for chunk j (embs from state[j]), as emit fns."""
                sl = slice(j * C, (j + 1) * C)
                actx = {}

                def s_stats():
                    emb_ct, emb_cfid = state.pop(j)
                    actx["emb_ct"], actx["emb_cfid"] = emb_ct, emb_cfid
                    sq_ct = sqp.tile([128, C], FR, tag="sqs")
                    nc.scalar.activation(out=sq_ct, in_=emb_ct, func=AF.Square)
                    sq_cfid = sqp.tile([128, C], FR, tag="sqs")
                    nc.scalar.activation(out=sq_cfid, in_=emb_cfid,
                                         func=AF.Square)
                    st36 = psmall.tile([36, C], F32, tag="st", bufs=1,
                                       name="st36")
                    nc.tensor.matmul(st36[0:36, :], hotabws[:, 0:36], emb_ct,
                                     start=True, stop=False)
                    nc.tensor.matmul(st36[0:36, :], hotabws[:, 36:72], emb_cfid,
                                     start=False, stop=False)
                    nc.tensor.matmul(st36[0:36, :], hotabws[:, 72:108], sq_ct,
                                     start=False, stop=False)
                    nc.tensor.matmul(st36[0:36, :], hotabws[:, 108:144], sq_cfid,
                                     start=False, stop=True)
                    actx["st36"] = st36

                def s_ab():
                    # folded l2norm+LN: A = rsqrt(ss*(1/64+eps) - mu^2),
                    # B = mu*A; then replicate to rows {0,32,64,96}+4 via PE.
                    st36 = actx.pop("st36")
                    mu4 = tinyp.tile([4, C], FR, tag="t4c")
                    nc.vector.tensor_scalar_mul(mu4, st36[0:4, :], 1.0 / 64)
                    musq = tinyp.tile([4, C], FR, tag="t4c")
                    nc.vector.tensor_mul(musq, mu4, mu4)
                    apre = tinyp.tile([4, C], FR, tag="t4c")
                    nc.vector.scalar_tensor_tensor(
                        apre, st36[32:36, :], 1.0 / 64 + LN_EPS, musq,
                        op0=ALU.mult, op1=ALU.subtract)
                    asq = tinyp.tile([4, C], FR, tag="t4c")
                    nc.scalar.activation(out=asq, in_=apre, func=AF.Sqrt)
                    ab36 = tmpp.tile([36, C], FR, tag="ab36", bufs=2)
                    nc.vector.reciprocal(ab36[0:4, :], asq)
                    nc.vector.tensor_mul(ab36[32:36, :], mu4, ab36[0:4, :])
                    abp = pbig.tile([128, C], F32, tag="mm", name="abp")
                    nc.tensor.matmul(abp[:, :], abrepls[:, :], ab36,
                                     start=True, stop=True)
                    ab4 = tmpp.tile([128, C], FR, tag="ab4", bufs=2)
                    nc.vector.tensor_copy(ab4, abp[:, :])
                    actx["ab4"] = ab4

                def s_xn():
                    # 4 concurrent row-tiled K=4 broadcasts -> A/B [128, C]
                    ab4 = actx.pop("ab4")
                    bps = []
                    for i, (r0, cs) in enumerate([(0, slice(0, 128)),
                                                  (32, slice(0, 128)),
                                                  (64, slice(128, 256)),
                                                  (96, slice(128, 256))]):
                        bp = pbig.tile([128, C], F32, tag="mm", name="bp")
                        nc.tensor.matmul(bp[:, :], pickqs[r0:r0 + 4, cs],
                                         ab4[r0:r0 + 4, :],
                                         start=True, stop=True,
                                         tile_position=(r0, 0))
                        bps.append(bp)
                    for pk, (ekey, ia, ib, tag) in enumerate(
                            [("emb_ct", 0, 1, "ct"), ("emb_cfid", 2, 3, "cfid")]):
                        tmp2 = tmpp.tile([128, C], FR, tag="tmp")
                        nc.vector.tensor_mul(tmp2, actx[ekey], bps[ia][:, :])
                        xn = xnp.tile([128, C], FR, tag="xn" + tag, name="xn")
                        nc.vector.tensor_sub(xn, tmp2, bps[ib][:, :])
                        actx["xn" + tag] = xn

                def s_qg(p):
                    def emit():
                        xn_p = actx["xnct" if p == 0 else "xncfid"]
                        qt = pbig.tile([128, C], F32, tag="mm", name="qt")
                        nc.tensor.matmul(qt[:, :], g22s[0:64, :],
                                         xn_p[0:64, :], start=True, stop=True,
                                         tile_position=(0, 0))
                        qb = pbig.tile([128, C], F32, tag="mm", name="qb")
                        nc.tensor.matmul(qb[:, :], g22s[64:128, :],
                                         xn_p[64:128, :], start=True,
                                         stop=True, tile_position=(64, 0))
                        actx["qg"] = (qt, qb)
                    return emit

                def s_pr(p):
                    def emit():
                        qt, qb = actx.pop("qg")
                        prs = []
                        for hi, qg_t in enumerate((qt, qb)):
                            for nk in ("xnct", "xncfid"):
                                pr = tmpp.tile([128, C], FR, tag="pr",
                                               name="pr", bufs=4)
                                nc.vector.tensor_mul(pr, qg_t[:, :], actx[nk])
                                prs.append(pr)
                        actx["prs"] = prs
                    return emit

                def s_dots(p):
                    def emit():
                        prs = actx.pop("prs")
                        if p == 0:
                            actx["s16"] = psmall.tile([16, C], F32, tag="s16",
                                                      bufs=1, name="s16")
                        s16 = actx["s16"]
                        for i, pr in enumerate(prs):
                            b = 4 * p + i
                            nc.tensor.matmul(s16[:, :],
                                             hotqs[:, 16 * b:16 * (b + 1)],
                                             pr, start=(p == 0 and i == 0),
                                             stop=(p == 1 and i == 3))
                    return emit

                def s_soft1():
                    s16 = actx.pop("s16")
                    e16 = tmpp.tile([16, C], FR, tag="e16", bufs=2)
                    nc.scalar.activation(out=e16, in_=s16[:, :], func=AF.Exp)
                    soft = psmall.tile([40, C], F32, tag="soft", bufs=1,
                                       name="soft")
                    nc.tensor.matmul(soft[0:4, :], sq16s[:, :], e16,
                                     start=True, stop=True)
                    tse = tinyp.tile([4, C], FR, tag="t4c")
                    nc.vector.tensor_scalar_mul(tse, soft[0:4, :], 4.0)
                    rr4 = tinyp.tile([4, C], FR, tag="t4c")
                    nc.vector.reciprocal(rr4, tse)
                    actx["e16"], actx["rr4"] = e16, rr4

                def s_soft2():
                    rrep = psmall.tile([40, C], F32, tag="soft", bufs=1,
                                       name="rrep")
                    nc.tensor.matmul(rrep[0:16, :], rep16s[:, :],
                                     actx["rr4"], start=True, stop=True)
                    w16 = tmpp.tile([16, C], FR, tag="w16", bufs=2)
                    nc.vector.tensor_mul(w16, actx.pop("e16"), rrep[0:16, :])
                    actx["w16"] = w16

                def s_out1():
                    a440 = psmall.tile([40, C], F32, tag="soft", bufs=1,
                                       name="a440")
                    nc.tensor.matmul(a440[:, :], nsqs[:, :], actx.pop("w16"),
                                     start=True, stop=True)
                    a4s = tmpp.tile([40, C], FR, tag="a4s", bufs=2)
                    nc.vector.tensor_copy(a4s, a440[:, :])
                    actx["a4s"] = a4s

                def s_out2():
                    a4s = actx.pop("a4s")
                    arp_ct = pbig.tile([128, C], F32, tag="mm", name="arp_ct")
                    nc.tensor.matmul(arp_ct[:, :], pickrs[0:4, 0:128],
                                     a4s[0:4, :], start=True, stop=True,
                                     tile_position=(0, 0))
                    arp_cf = pbig.tile([128, C], F32, tag="mm", name="arp_cf")
                    nc.tensor.matmul(arp_cf[:, :], pickrs[32:36, 128:256],
                                     a4s[32:36, :], start=True, stop=True,
                                     tile_position=(32, 0))
                    zp_ct = tmpp.tile([128, C], FR, tag="pr", name="zp_ct",
                                      bufs=4)
                    nc.vector.tensor_mul(zp_ct, actx["xnct"], arp_ct[:, :])
                    zp_cf = tmpp.tile([128, C], FR, tag="pr", name="zp_cf",
                                      bufs=4)
                    nc.vector.tensor_mul(zp_cf, actx["xncfid"], arp_cf[:, :])
                    zsum = tmpp.tile([128, C], FR, tag="tmp", name="zsum")
                    nc.vector.tensor_add(zsum, zp_ct, zp_cf)
                    actx["zsum"] = zsum

                def s_out3():
                    fps = pbig.tile([128, C], F32, tag="mm")
                    nc.tensor.matmul(fps[0:64, :], wv2s[:, :],
                                     actx.pop("zsum"), start=True, stop=True)
                    out_sb = outp.tile([64, C], F32, tag="osb")
                    nc.vector.tensor_copy(out_sb, fps[0:64, :])
                    nc.sync.dma_start(out=out[:, sl], in_=out_sb)

                return [s_stats, s_ab, s_xn,
                        s_qg(0), s_pr(0), s_dots(0),
                        s_qg(1), s_pr(1), s_dots(1),
                        s_soft1, s_soft2, s_out1, s_out2, s_out3]

            import os
            skip_attn = bool(int(os.environ.get("K_SKIP_ATTN", "0")))
            skip_mlp = bool(int(os.environ.get("K_SKIP_MLP", "0")))

            def emit_all():
                for it in range(nch + 1):
                    units = make_units(it) if it < nch else []
                    stages = (make_stages(it - 1) if it >= 1 else [])
                    if skip_attn:
                        stages = []
                        if it >= 1:
                            state.pop(it - 1, None)
                    if skip_mlp:
                        units = units[:1]  # loads only
                        stages = []
                    # weave: distribute stages evenly among units
                    n_u, n_s = len(units), len(stages)
                    si = 0
                    for ui, u in enumerate(units):
                        u()
                        want = ((ui + 1) * n_s) // max(n_u, 1)
                        while si < want:
                            stages[si]()
                            si += 1
                    while si < n_s:
                        stages[si]()
                        si += 1

            if repeat == 1:
                emit_all()
            else:
                with tc.For_i(0, repeat, 1):
                    emit_all()

    nc.finalize()
    return nc


def _get_nc(nch=NCH0):
    key = ("nc", nch)
    if key not in _CACHE:
        _CACHE[key] = _build_nc(nch=nch)
    return _CACHE[key]


def _l2norm_rows(x):
    n = np.sqrt(np.einsum("ij,ij->i", x, x, dtype=np.float64))
    n = np.maximum(n, 1e-12).astype(np.float32)
    return x / n[:, None]


def _host_prep(inputs, nch=None):
    """Dedup + pre-gather + normalize.  Returns (in_maps, inv, nch)."""
    f32 = np.float32
    seq = np.asarray(inputs["seq_modify"])
    seq = np.where(seq == NUM_ITEM, 0, seq).astype(np.int64)  # [1024, 50]
    uniq, inv = np.unique(seq.ravel(), return_inverse=True)
    if nch is None:
        nch = max(1, -(-uniq.size // (N_CORES * C)))
    T = C * nch
    total = N_CORES * T
    assert uniq.size <= total
    idx_full = np.zeros(total, np.int64)
    idx_full[:uniq.size] = uniq

    cf_full = _l2norm_rows(np.asarray(inputs["content_feature"], dtype=f32))
    tf_full = _l2norm_rows(np.asarray(inputs["text_feature"], dtype=f32))
    cff_full = np.asarray(inputs["cf_feature"], dtype=f32)
    ide_full = np.asarray(inputs["item_embeddings"], dtype=f32)

    c_w3 = np.asarray(inputs["c_w3"], dtype=f32)   # [64, 256]
    t_w3 = np.asarray(inputs["t_w3"], dtype=f32)
    cw3T = np.ascontiguousarray(c_w3.T)            # [256, 64]
    tw3T = np.ascontiguousarray(t_w3.T)
    w3p = np.zeros((128, 4, 128), f32)
    w3p[:, 0, 0:64] = cw3T[0:128]
    w3p[:, 1, 0:64] = cw3T[128:256]
    w3p[:, 2, 64:128] = tw3T[0:128]
    w3p[:, 3, 64:128] = tw3T[128:256]
    cf_w = np.asarray(inputs["cf_w"], dtype=f32)
    cfwp = np.zeros((64, 128), f32)
    cfwp[:, 0:64] = cf_w.T
    wq = np.asarray(inputs["wq"], dtype=f32)
    wk = np.asarray(inputs["wk"], dtype=f32)
    wv = np.asarray(inputs["wv"], dtype=f32)
    G = (wq.T @ wk) * (D ** -0.5)
    G2 = np.concatenate([G, G], axis=1)            # [64, 128]
    g22 = np.concatenate([G2, G2], axis=0)         # [128, 128]
    wv2 = np.concatenate([wv.T, wv.T], axis=0)     # [128, 64]
    b3 = np.concatenate([np.asarray(inputs["c_b3"], dtype=f32),
                         np.asarray(inputs["t_b3"], dtype=f32)])[:, None]

    shared = dict(
        cw1=np.ascontiguousarray(np.asarray(inputs["c_w1"], dtype=f32).T
                                 .astype(_bf())),
        cw2=np.ascontiguousarray(np.asarray(inputs["c_w2"], dtype=f32).T
                                 .astype(_bf())),
        tw1=np.ascontiguousarray(np.asarray(inputs["t_w1"], dtype=f32).T
                                 .astype(_bf())),
        tw2=np.ascontiguousarray(np.asarray(inputs["t_w2"], dtype=f32).T
                                 .astype(_bf())),
        w3p=w3p, cfwp=cfwp, g22=g22, wv2=wv2,
        b1c=np.asarray(inputs["c_b1"], dtype=f32).reshape(KC, 128),
        b2c=np.asarray(inputs["c_b2"], dtype=f32).reshape(2, 128),
        b1t=np.asarray(inputs["t_b1"], dtype=f32).reshape(KT_, 128),
        b2t=np.asarray(inputs["t_b2"], dtype=f32).reshape(2, 128),
        b3=b3,
        bcf=np.asarray(inputs["cf_b"], dtype=f32)[:, None],
        **_build_consts(),
    )

    in_maps = []
    for c in range(N_CORES):
        idx = idx_full[c * T:(c + 1) * T]
        m = dict(shared)
        m["xc"] = np.ascontiguousarray(cf_full[idx].T.astype(_bf()))
        m["xt"] = np.ascontiguousarray(tf_full[idx].T.astype(_bf()))
        m["xcf"] = np.ascontiguousarray(cff_full[idx].T)
        m["xid"] = np.ascontiguousarray(ide_full[idx].T)
        in_maps.append(m)
    return in_maps, inv, nch


def _get_runner(nc=None, key="runner", nch=NCH0):
    """Cached jitted shard_map runner over 8 cores (mirrors
    bass2jax.run_bass_via_pjrt but reuses one jit so repeat calls skip
    retracing)."""
    if key in _CACHE:
        return _CACHE[key]
    import jax
    from jax.sharding import Mesh, PartitionSpec
    try:
        from jax.experimental.shard_map import shard_map
    except ImportError:
        from jax.shard_map import shard_map
    from concourse import bass2jax, mybir

    if nc is None:
        nc = _get_nc(nch)
    bass2jax.install_neuronx_cc_hook()
    partition_name = (nc.partition_id_tensor.name
                      if nc.partition_id_tensor else None)
    in_names, out_names, out_avals, zero_shapes = [], [], [], []
    for alloc in nc.m.functions[0].allocations:
        if not isinstance(alloc, mybir.MemoryLocationSet):
            continue
        name = alloc.memorylocations[0].name
        if alloc.kind == "ExternalInput":
            if name != partition_name:
                in_names.append(name)
        elif alloc.kind == "ExternalOutput":
            out_names.append(name)
            shape = tuple(alloc.tensor_shape)
            dtype = mybir.dt.np(alloc.dtype)
            out_avals.append(jax.core.ShapedArray(shape, dtype))
            zero_shapes.append((shape, dtype))
    n_params = len(in_names)
    full_in_names = list(in_names) + list(out_names)
    if partition_name is not None:
        full_in_names.append(partition_name)

    def _body(*args):
        operands = list(args)
        if partition_name is not None:
            operands.append(bass2jax.partition_id_tensor())
        outs = bass2jax._bass_exec_p.bind(
            *operands,
            out_avals=tuple(out_avals),
            in_names=tuple(full_in_names),
            out_names=tuple(out_names),
            lowering_input_output_aliases=(),
            sim_require_finite=True,
            sim_require_nnan=True,
            nc=nc,
        )
        return tuple(outs)

    devices = jax.devices()[:N_CORES]
    mesh = Mesh(np.asarray(devices), ("core",))
    n_outs = len(out_names)
    in_specs = (PartitionSpec("core"),) * (n_params + n_outs)
    out_specs = (PartitionSpec("core"),) * n_outs
    sharded = jax.jit(
        shard_map(_body, mesh=mesh, in_specs=in_specs, out_specs=out_specs,
                  check_rep=False),
        keep_unused=True,
    )
    runner = (sharded, in_names, out_names, zero_shapes, mesh)
    _CACHE[key] = runner
    return runner


def _run_device(in_maps, nch):
    sharded, in_names, out_names, zero_shapes, _ = _get_runner(
        key=("runner", nch), nch=nch)
    concat_in = [
        np.concatenate([np.asarray(in_maps[c][n]) for c in range(N_CORES)],
                       axis=0)
        for n in in_names
    ]
    concat_zeros = [np.zeros((N_CORES * s[0], *s[1:]), d)
                    for (s, d) in zero_shapes]
    out_arrs = sharded(*concat_in, *concat_zeros)
    return np.asarray(out_arrs[out_names.index("out")])


def kernel(**inputs):
    in_maps, inv, nch = _host_prep(inputs)
    out_cat = _run_device(in_maps, nch)     # [8*64, T]
    T = C * nch
    f_all = (out_cat.reshape(N_CORES, 64, T)
             .transpose(0, 2, 1).reshape(N_CORES * T, 64))
    return f_all[inv].reshape(B, S, D).astype(np.float32)
